# revision 13
# baseline (speedup 1.0000x reference)
import os
import sys

sys.path.insert(0, "/opt/trn_rl_repo")

import numpy as np

import concourse.bacc as bacc
import concourse.bass as bass
import concourse.mybir as mybir
import concourse.tile as tile
from concourse.bass_utils import run_bass_kernel_spmd
from concourse.masks import make_identity

FP32 = mybir.dt.float32
BF16 = mybir.dt.bfloat16
I32 = mybir.dt.int32

P = 128
NCORES = 8
B = 32
GP = 768            # padded slots per graph
GPC = 4             # graphs per core
SPC = GP * GPC      # 3072 slots per core
NT = SPC // P       # 24 node tiles per core
NSLOT = NCORES * SPC  # 24576
N = 20000
EMB = 768
L = 512

LAST_EXEC_NS = None

# table column layouts (bf16): [h | a_src | a_dst]
C1H, C1A = 1024, 8   # L1: 8 groups x 128
C2H, C2A = 512, 2    # L2: 2 groups x 256
C3H, C3A = 256, 2    # L3: 2 groups x 128
C1 = C1H + 2 * C1A   # 1040
C2 = C2H + 2 * C2A   # 516
C3 = C3H + 2 * C3A   # 260


def _fold_aug(Wcat, att_srcs, att_dsts, heads, cdim):
    # Wcat [cin, G*cdim]; att lists per conv of [heads, cdim]
    cin = Wcat.shape[0]
    G = Wcat.shape[1] // cdim
    asrc = np.zeros((cin, G), np.float32)
    adst = np.zeros((cin, G), np.float32)
    nconv = G // heads
    for conv in range(nconv):
        Wc = Wcat[:, conv * heads * cdim:(conv + 1) * heads * cdim].reshape(cin, heads, cdim)
        asrc[:, conv * heads:(conv + 1) * heads] = np.einsum("ihc,hc->ih", Wc, att_srcs[conv])
        adst[:, conv * heads:(conv + 1) * heads] = np.einsum("ihc,hc->ih", Wc, att_dsts[conv])
    return np.concatenate([Wcat, asrc, adst], axis=1).astype(np.float32)


def preprocess(x, edge_index, batch, params):
    x = np.asarray(x, np.float32)
    ei = np.asarray(edge_index)
    batch = np.asarray(batch).astype(np.int64)
    n = x.shape[0]

    sizes = np.bincount(batch, minlength=B)
    assert sizes.max() <= GP, f"graph size {sizes.max()} > {GP}"
    starts = np.zeros(B, np.int64)
    starts[1:] = np.cumsum(sizes)[:-1]
    # batch is sorted; slot of node i = batch[i]*GP + (i - start[batch[i]])
    slot = batch * GP + (np.arange(n) - starts[batch])

    src = np.concatenate([ei[0], np.arange(n)]).astype(np.int64)
    dst = np.concatenate([ei[1], np.arange(n)]).astype(np.int64)
    ss = slot[src]
    ds = slot[dst]

    order = np.argsort(ds, kind="stable")
    ss, ds = ss[order], ds[order]
    core = ds // SPC
    tloc = (ds % SPC) // P

    # count per (core, tile)
    cnt = np.zeros((NCORES, NT), np.int64)
    np.add.at(cnt, (core, tloc), 1)
    max_cpt = int(np.ceil(cnt.max() / P))
    nchunk = NT * max_cpt

    srcg = np.zeros((NCORES, nchunk * P), np.int32)
    dstg = np.zeros((NCORES, nchunk * P), np.int32)
    dstl = np.full((NCORES, nchunk * P), 300.0, np.float32)

    # edges are sorted by ds -> contiguous runs per (core, tile)
    # compute position of each edge within its (core,tile) run
    key = core * NT + tloc
    # start index of each key run
    run_start = np.zeros(NCORES * NT, np.int64)
    ksorted_counts = np.bincount(key, minlength=NCORES * NT)
    run_start[1:] = np.cumsum(ksorted_counts)[:-1]
    pos_in_run = np.arange(len(ss)) - run_start[key]
    for c in range(NCORES):
        m = core == c
        pp = (tloc[m] * (max_cpt * P) + pos_in_run[m]).astype(np.int64)
        srcg[c, pp] = ss[m]
        dstg[c, pp] = ds[m]
        dstl[c, pp] = (ds[m] % P).astype(np.float32)

    # [128, nchunk] column-major per chunk: element e of chunk k at [e, k]
    def cols(a):
        return np.ascontiguousarray(a.reshape(NCORES, nchunk, P).transpose(0, 2, 1))

    srcg_c, dstg_c, dstl_c = cols(srcg), cols(dstg), cols(dstl)

    # x slices transposed [256, SPC] per core
    xT = np.zeros((NCORES, x.shape[1], SPC), np.float32)
    for c in range(NCORES):
        m = (slot >= c * SPC) & (slot < (c + 1) * SPC)
        xT[c][:, slot[m] - c * SPC] = x[m].T

    convs = [params["convA"], params["convB"], params["convC"], params["convD"]]
    W1cat = np.concatenate([np.asarray(p["W"], np.float32) for p in convs], axis=1)
    w1aug = _fold_aug(W1cat, [np.asarray(p["att_src"], np.float32) for p in convs],
                      [np.asarray(p["att_dst"], np.float32) for p in convs], 2, 128)
    p2 = params["conv2"]
    w2aug = _fold_aug(np.asarray(p2["W"], np.float32), [np.asarray(p2["att_src"], np.float32)],
                      [np.asarray(p2["att_dst"], np.float32)], 2, 256)
    p3 = params["conv3"]
    w3aug = _fold_aug(np.asarray(p3["W"], np.float32), [np.asarray(p3["att_src"], np.float32)],
                      [np.asarray(p3["att_dst"], np.float32)], 2, 128)

    b1 = np.concatenate([np.asarray(p["b"], np.float32) for p in convs])
    b1t = np.tile(b1[None, :], (P, 1)).astype(np.float32)
    b2t = np.tile(np.asarray(p2["b"], np.float32)[None, :], (P, 1))
    b3t = np.tile(np.asarray(p3["b"], np.float32)[None, :], (P, 1))

    iota = np.tile(np.arange(P, dtype=np.float32)[None, :], (P, 1))

    onehot = np.zeros((NCORES, SPC, GPC), np.float32)
    padmask = np.full((NCORES, P, SPC), -1e30, np.float32)
    for c in range(NCORES):
        for gl in range(GPC):
            g = c * GPC + gl
            sz = int(sizes[g])
            onehot[c, gl * GP:gl * GP + sz, gl] = 1.0
            padmask[c, :, gl * GP:gl * GP + sz] = 0.0

    onesel = np.zeros((P, GPC * GPC), np.float32)
    for g in range(GPC):
        onesel[:, g * GPC + g] = 1.0

    return dict(slot=slot, sizes=sizes, max_cpt=max_cpt, nchunk=nchunk,
                srcg=srcg_c, dstg=dstg_c, dstl=dstl_c, xT=xT,
                w1aug=w1aug, w2aug=w2aug, w3aug=w3aug,
                b1t=b1t, b2t=b2t, b3t=b3t, iota=iota,
                onehot=onehot, padmask=padmask, onesel=onesel)


def build_program(max_cpt):
    nchunk = NT * max_cpt
    nc = bacc.Bacc("TRN2", num_devices=NCORES)

    xT = nc.dram_tensor("xT", [256, SPC], FP32, kind="ExternalInput")
    w1 = nc.dram_tensor("w1", [256, C1], FP32, kind="ExternalInput")
    w2 = nc.dram_tensor("w2", [512, C2], FP32, kind="ExternalInput")
    w3 = nc.dram_tensor("w3", [256, C3], FP32, kind="ExternalInput")
    b1t = nc.dram_tensor("b1t", [P, 512], FP32, kind="ExternalInput")
    b2t = nc.dram_tensor("b2t", [P, 256], FP32, kind="ExternalInput")
    b3t = nc.dram_tensor("b3t", [P, 128], FP32, kind="ExternalInput")
    iota = nc.dram_tensor("iota", [P, P], FP32, kind="ExternalInput")
    srcg = nc.dram_tensor("srcg", [P, nchunk], I32, kind="ExternalInput")
    dstg = nc.dram_tensor("dstg", [P, nchunk], I32, kind="ExternalInput")
    dstl = nc.dram_tensor("dstl", [P, nchunk], FP32, kind="ExternalInput")
    onehot = nc.dram_tensor("onehot", [SPC, GPC], FP32, kind="ExternalInput")
    padmask = nc.dram_tensor("padmask", [P, SPC], FP32, kind="ExternalInput")
    seq = nc.dram_tensor("seq", [GPC * L, EMB], FP32, kind="ExternalInput")
    onesel = nc.dram_tensor("onesel", [P, GPC * GPC], FP32, kind="ExternalInput")

    gsum = nc.dram_tensor("gsum", [GPC, 128], FP32, kind="ExternalOutput")
    gmax = nc.dram_tensor("gmax", [P, GPC], FP32, kind="ExternalOutput")
    seqsum = nc.dram_tensor("seqsum", [GPC, EMB], FP32, kind="ExternalOutput")

    slice1 = nc.dram_tensor("slice1", [SPC, C1], BF16)
    slice2 = nc.dram_tensor("slice2", [SPC, C2], BF16)
    slice3 = nc.dram_tensor("slice3", [SPC, C3], BF16)
    table1 = nc.dram_tensor("table1", [NSLOT, C1], BF16, addr_space="Shared")
    table2 = nc.dram_tensor("table2", [NSLOT, C2], BF16, addr_space="Shared")
    table3 = nc.dram_tensor("table3", [NSLOT, C3], BF16, addr_space="Shared")

    RG = [list(range(NCORES))]
    AOT = mybir.AluOpType

    with tile.TileContext(nc) as tc:
        with tc.tile_pool(name="const", bufs=1) as cp:
            ident = cp.tile([P, P], FP32)
            make_identity(nc, ident[:])
            iota_sb = cp.tile([P, P], FP32)
            nc.sync.dma_start(out=iota_sb[:], in_=iota[:])
            srcg_sb = cp.tile([P, nchunk], I32)
            nc.sync.dma_start(out=srcg_sb[:], in_=srcg[:])
            dstg_sb = cp.tile([P, nchunk], I32)
            nc.sync.dma_start(out=dstg_sb[:], in_=dstg[:])
            dstl_sb = cp.tile([P, nchunk], FP32)
            nc.sync.dma_start(out=dstl_sb[:], in_=dstl[:])
            b1_sb = cp.tile([P, 512], FP32)
            nc.sync.dma_start(out=b1_sb[:], in_=b1t[:])
            b2_sb = cp.tile([P, 256], FP32)
            nc.sync.dma_start(out=b2_sb[:], in_=b2t[:])
            b3_sb = cp.tile([P, 128], FP32)
            nc.sync.dma_start(out=b3_sb[:], in_=b3t[:])
            xT_sb = [cp.tile([P, SPC], FP32, tag=f"xT{k}", name=f"xT{k}") for k in range(2)]
            for k in range(2):
                nc.sync.dma_start(out=xT_sb[k][:], in_=xT[k * P:(k + 1) * P, :])
            w1_sb = [cp.tile([P, C1], FP32, tag=f"w1_{k}", name=f"w1s{k}") for k in range(2)]
            for k in range(2):
                nc.sync.dma_start(out=w1_sb[k][:], in_=w1[k * P:(k + 1) * P, :])
            w2_sb = [cp.tile([P, C2], FP32, tag=f"w2_{k}", name=f"w2s{k}") for k in range(4)]
            for k in range(4):
                nc.sync.dma_start(out=w2_sb[k][:], in_=w2[k * P:(k + 1) * P, :])
            w3_sb = [cp.tile([P, C3], FP32, tag=f"w3_{k}", name=f"w3s{k}") for k in range(2)]
            for k in range(2):
                nc.sync.dma_start(out=w3_sb[k][:], in_=w3[k * P:(k + 1) * P, :])
            x2T_sb = [cp.tile([P, SPC], FP32, tag=f"x2T{k}", name=f"x2T{k}") for k in range(4)]
            x3T_sb = [cp.tile([P, SPC], FP32, tag=f"x3T{k}", name=f"x3T{k}") for k in range(2)]
            hfT_sb = cp.tile([P, SPC], FP32)

            # ---------------- Phase A: h1_aug = x @ w1aug ----------------
            with tc.tile_pool(name="psA", bufs=2, space="PSUM") as psA, \
                 tc.tile_pool(name="sbA", bufs=3) as sbA:
                for t in range(NT):
                    ph = [psA.tile([P, 512], FP32, tag="pha", name="pha"), psA.tile([P, 512], FP32, tag="phb", name="phb"), psA.tile([P, 16], FP32, tag="phc", name="phc")]
                    fr = [(0, 512), (512, 1024), (1024, 1040)]
                    for pi, (f0, f1) in enumerate(fr):
                        for k in range(2):
                            nc.tensor.matmul(ph[pi][:, :f1 - f0],
                                             lhsT=xT_sb[k][:, t * P:(t + 1) * P],
                                             rhs=w1_sb[k][:, f0:f1],
                                             start=(k == 0), stop=(k == 1))
                    hb = sbA.tile([P, C1], BF16)
                    for pi, (f0, f1) in enumerate(fr):
                        nc.vector.tensor_copy(out=hb[:, f0:f1], in_=ph[pi][:, :f1 - f0])
                    nc.sync.dma_start(out=slice1[t * P:(t + 1) * P, :], in_=hb[:])

            with tc.tile_pool(name="dr1", bufs=1, space="DRAM") as dr1:
                nc.gpsimd.collective_compute("AllGather", AOT.bypass, replica_groups=RG,
                                             ins=[slice1[:]], outs=[table1[:]])

            # ---------------- Phase B: edge L1 -> x2 -> h2_aug ----------------
            def edge_phase(tbl, CH, CA, ngroups, gw, pn_shapes, bias_sb, relu,
                           out_cb):
                # returns per-tile node-partition output [P, ngroups//2*gw]
                with tc.tile_pool(name="psE", bufs=2, space="PSUM") as psE, \
                     tc.tile_pool(name="psD", bufs=1, space="PSUM") as psD, \
                     tc.tile_pool(name="sbE", bufs=4) as sbE:
                    CT = CH + 2 * CA
                    for t in range(NT):
                        pns = [psE.tile([P, s], FP32, tag=f"pn{i}", name=f"pn{i}")
                               for i, s in enumerate(pn_shapes)]
                        pd = psD.tile([P, CA], FP32, tag="pd")
                        last = max_cpt - 1
                        for j in range(max_cpt):
                            kk = t * max_cpt + j
                            hbuf = sbE.tile([P, CT], BF16, tag="hbuf")
                            nc.gpsimd.indirect_dma_start(
                                out=hbuf[:], out_offset=None, in_=tbl[:, :],
                                in_offset=bass.IndirectOffsetOnAxis(ap=srcg_sb[:, kk:kk + 1], axis=0))
                            adst = sbE.tile([P, CA], BF16, tag="adst")
                            nc.gpsimd.indirect_dma_start(
                                out=adst[:], out_offset=None, in_=tbl[:, 0:CA],
                                in_offset=bass.IndirectOffsetOnAxis(ap=dstg_sb[:, kk:kk + 1], axis=0),
                                element_offset=CH + CA)
                            sv = sbE.tile([P, CA], FP32, tag="sv")
                            nc.vector.tensor_tensor(out=sv[:], in0=hbuf[:, CH:CH + CA],
                                                    in1=adst[:], op=AOT.add)
                            ev = sbE.tile([P, CA], FP32, tag="ev")
                            nc.vector.tensor_scalar(out=ev[:], in0=sv[:], scalar1=0.2,
                                                    scalar2=None, op0=AOT.mult)
                            nc.vector.tensor_tensor(out=ev[:], in0=ev[:], in1=sv[:], op=AOT.max)
                            pbf = sbE.tile([P, CA], BF16, tag="pbf")
                            nc.scalar.activation(pbf[:], ev[:], mybir.ActivationFunctionType.Exp)
                            S = sbE.tile([P, P], BF16, tag="S")
                            nc.vector.tensor_tensor(
                                out=S[:], in0=iota_sb[:],
                                in1=dstl_sb[:, kk:kk + 1].to_broadcast([P, P]),
                                op=AOT.is_equal)
                            for g in range(ngroups):
                                nc.vector.tensor_tensor(
                                    out=hbuf[:, g * gw:(g + 1) * gw],
                                    in0=hbuf[:, g * gw:(g + 1) * gw],
                                    in1=pbf[:, g:g + 1].to_broadcast([P, gw]),
                                    op=AOT.mult)
                            nc.tensor.matmul(pd[:], lhsT=S[:], rhs=pbf[:],
                                             start=(j == 0), stop=(j == last))
                            gpb = pn_shapes[0] // gw  # groups per psum tile
                            for g in range(ngroups):
                                pt = pns[g // gpb]
                                off = (g % gpb) * gw
                                nc.tensor.matmul(pt[:, off:off + gw], lhsT=S[:],
                                                 rhs=hbuf[:, g * gw:(g + 1) * gw],
                                                 start=(j == 0), stop=(j == last))
                        # epilogue: alpha-normalize + head mean + bias (+relu)
                        den = sbE.tile([P, CA], FP32, tag="den")
                        nc.vector.tensor_scalar(out=den[:], in0=pd[:], scalar1=1e-16, scalar2=None, op0=AOT.add)
                        rden = sbE.tile([P, CA], FP32, tag="rden")
                        nc.vector.reciprocal(rden[:], den[:])
                        nc.vector.tensor_scalar(out=rden[:], in0=rden[:], scalar1=0.5,
                                                scalar2=None, op0=AOT.mult)
                        nconv = ngroups // 2
                        xo = sbE.tile([P, nconv * gw], FP32, tag="xo")
                        tmp = sbE.tile([P, gw], FP32, tag="tmp")
                        for conv in range(nconv):
                            g0, g1 = 2 * conv, 2 * conv + 1
                            gpb = pn_shapes[0] // gw

                            def pslice(g):
                                return pns[g // gpb][:, (g % gpb) * gw:(g % gpb) * gw + gw]

                            nc.vector.tensor_tensor(
                                out=tmp[:], in0=pslice(g0),
                                in1=rden[:, g0:g0 + 1].to_broadcast([P, gw]), op=AOT.mult)
                            nc.vector.tensor_tensor(
                                out=xo[:, conv * gw:(conv + 1) * gw], in0=pslice(g1),
                                in1=rden[:, g1:g1 + 1].to_broadcast([P, gw]), op=AOT.mult)
                            nc.vector.tensor_tensor(
                                out=xo[:, conv * gw:(conv + 1) * gw],
                                in0=xo[:, conv * gw:(conv + 1) * gw], in1=tmp[:], op=AOT.add)
                            nc.vector.tensor_tensor(
                                out=xo[:, conv * gw:(conv + 1) * gw],
                                in0=xo[:, conv * gw:(conv + 1) * gw],
                                in1=bias_sb[:, conv * gw:(conv + 1) * gw], op=AOT.add)
                            if relu:
                                nc.scalar.activation(xo[:, conv * gw:(conv + 1) * gw],
                                                     xo[:, conv * gw:(conv + 1) * gw],
                                                     mybir.ActivationFunctionType.Relu)
                        out_cb(t, xo, psE, sbE)

            # phase B body callback: transpose x2, compute h2_aug, write slice2
            with tc.tile_pool(name="psT", bufs=1, space="PSUM") as psT, \
                 tc.tile_pool(name="sbT", bufs=3) as sbT:

                def cb_b(t, xo, psE, sbE):
                    for fc in range(4):
                        ptr = psT.tile([P, P], FP32, tag="tr")
                        nc.tensor.transpose(out=ptr[:], in_=xo[:, fc * P:(fc + 1) * P],
                                            identity=ident[:])
                        nc.vector.tensor_copy(out=x2T_sb[fc][:, t * P:(t + 1) * P], in_=ptr[:])
                    pha = psT.tile([P, 512], FP32, tag="h2a")
                    phb = psT.tile([P, 4], FP32, tag="h2b")
                    for k in range(4):
                        nc.tensor.matmul(pha[:], lhsT=x2T_sb[k][:, t * P:(t + 1) * P],
                                         rhs=w2_sb[k][:, 0:512], start=(k == 0), stop=(k == 3))
                        nc.tensor.matmul(phb[:], lhsT=x2T_sb[k][:, t * P:(t + 1) * P],
                                         rhs=w2_sb[k][:, 512:516], start=(k == 0), stop=(k == 3))
                    hb2 = sbT.tile([P, C2], BF16, tag="hb2")
                    nc.vector.tensor_copy(out=hb2[:, 0:512], in_=pha[:])
                    nc.vector.tensor_copy(out=hb2[:, 512:516], in_=phb[:])
                    nc.sync.dma_start(out=slice2[t * P:(t + 1) * P, :], in_=hb2[:])

                edge_phase(table1, C1H, C1A, 8, 128, [512, 512], b1_sb, True, cb_b)

            nc.gpsimd.collective_compute("AllGather", AOT.bypass, replica_groups=RG,
                                         ins=[slice2[:]], outs=[table2[:]])

            # ---------------- Phase C: edge L2 -> x3 -> h3_aug ----------------
            with tc.tile_pool(name="psT3", bufs=1, space="PSUM") as psT3, \
                 tc.tile_pool(name="sbT3", bufs=3) as sbT3:

                def cb_c(t, xo, psE, sbE):
                    for fc in range(2):
                        ptr = psT3.tile([P, P], FP32, tag="tr3")
                        nc.tensor.transpose(out=ptr[:], in_=xo[:, fc * P:(fc + 1) * P],
                                            identity=ident[:])
                        nc.vector.tensor_copy(out=x3T_sb[fc][:, t * P:(t + 1) * P], in_=ptr[:])
                    pha = psT3.tile([P, C3], FP32, tag="h3a")
                    for k in range(2):
                        nc.tensor.matmul(pha[:], lhsT=x3T_sb[k][:, t * P:(t + 1) * P],
                                         rhs=w3_sb[k][:], start=(k == 0), stop=(k == 1))
                    hb3 = sbT3.tile([P, C3], BF16, tag="hb3")
                    nc.vector.tensor_copy(out=hb3[:], in_=pha[:])
                    nc.sync.dma_start(out=slice3[t * P:(t + 1) * P, :], in_=hb3[:])

                edge_phase(table2, C2H, C2A, 2, 256, [512], b2_sb, False, cb_c)

            nc.gpsimd.collective_compute("AllGather", AOT.bypass, replica_groups=RG,
                                         ins=[slice3[:]], outs=[table3[:]])

            # ---------------- Phase D: edge L3 -> h_f -> pooling ----------------
            with tc.tile_pool(name="psP", bufs=1, space="PSUM") as psP, \
                 tc.tile_pool(name="sbP", bufs=3) as sbP:
                ppool = psP.tile([GPC, 128], FP32, tag="pool")

                def cb_d(t, xo, psE, sbE):
                    oh = sbP.tile([P, GPC], FP32, tag="oh")
                    nc.sync.dma_start(out=oh[:], in_=onehot[t * P:(t + 1) * P, :])
                    nc.tensor.matmul(ppool[:], lhsT=oh[:], rhs=xo[:],
                                     start=(t == 0), stop=(t == NT - 1))
                    ptr = psE.tile([P, P], FP32, tag="trD")
                    nc.tensor.transpose(out=ptr[:], in_=xo[:], identity=ident[:])
                    nc.vector.tensor_copy(out=hfT_sb[:, t * P:(t + 1) * P], in_=ptr[:])

                edge_phase(table3, C3H, C3A, 2, 128, [256], b3_sb, False, cb_d)

                pm = sbP.tile([P, SPC], FP32, tag="pm")
                nc.sync.dma_start(out=pm[:], in_=padmask[:])
                nc.vector.tensor_tensor(out=hfT_sb[:], in0=hfT_sb[:], in1=pm[:], op=AOT.add)
                gm = sbP.tile([P, GPC], FP32, tag="gm")
                for g in range(GPC):
                    nc.vector.tensor_reduce(gm[:, g:g + 1], hfT_sb[:, g * GP:(g + 1) * GP],
                                            mybir.AxisListType.X, AOT.max)
                nc.sync.dma_start(out=gmax[:], in_=gm[:])
                gs = sbP.tile([GPC, 128], FP32, tag="gs")
                nc.vector.tensor_copy(out=gs[:], in_=ppool[:])
                nc.sync.dma_start(out=gsum[:], in_=gs[:])

            # ---------------- Phase E: seq mean ----------------
            with tc.tile_pool(name="psS", bufs=1, space="PSUM") as psS, \
                 tc.tile_pool(name="sbS", bufs=3) as sbS:
                os_sb = sbS.tile([P, GPC * GPC], FP32, tag="os")
                nc.sync.dma_start(out=os_sb[:], in_=onesel[:])
                psq = [psS.tile([GPC, 384], FP32, tag=f"sq{i}", name=f"psq{i}") for i in range(2)]
                nchk = GPC * L // P  # 16
                for ci in range(nchk):
                    st = sbS.tile([P, EMB], FP32, tag="st")
                    nc.sync.dma_start(out=st[:], in_=seq[ci * P:(ci + 1) * P, :])
                    g = ci // (L // P)
                    for h in range(2):
                        nc.tensor.matmul(psq[h][:], lhsT=os_sb[:, g * GPC:(g + 1) * GPC],
                                         rhs=st[:, h * 384:(h + 1) * 384],
                                         start=(ci == 0), stop=(ci == nchk - 1))
                sq = sbS.tile([GPC, EMB], FP32, tag="sq")
                nc.vector.tensor_copy(out=sq[:, 0:384], in_=psq[0][:])
                nc.vector.tensor_copy(out=sq[:, 384:768], in_=psq[1][:])
                nc.sync.dma_start(out=seqsum[:], in_=sq[:])

    nc.compile()
    return nc


def _mlp(x, p):
    h = x @ np.asarray(p["W1"], np.float32) + np.asarray(p["b1"], np.float32)
    h = np.maximum(h, 0.0)
    return h @ np.asarray(p["W2"], np.float32) + np.asarray(p["b2"], np.float32)


def _sigmoid(v):
    return 1.0 / (1.0 + np.exp(-v))


def kernel(x, edge_index, edge_attr, batch, seq_hidden, params):
    global LAST_EXEC_NS
    pre = preprocess(x, edge_index, batch, params)
    nc = build_program(pre["max_cpt"])

    seq_hidden = np.asarray(seq_hidden, np.float32)
    in_maps = []
    for c in range(NCORES):
        in_maps.append({
            "xT": np.ascontiguousarray(pre["xT"][c]),
            "w1": pre["w1aug"], "w2": pre["w2aug"], "w3": pre["w3aug"],
            "b1t": pre["b1t"], "b2t": pre["b2t"], "b3t": pre["b3t"],
            "iota": pre["iota"],
            "srcg": np.ascontiguousarray(pre["srcg"][c]),
            "dstg": np.ascontiguousarray(pre["dstg"][c]),
            "dstl": np.ascontiguousarray(pre["dstl"][c]),
            "onehot": np.ascontiguousarray(pre["onehot"][c]),
            "padmask": np.ascontiguousarray(pre["padmask"][c]),
            "seq": np.ascontiguousarray(
                seq_hidden[c * GPC:(c + 1) * GPC].reshape(GPC * L, EMB)),
            "onesel": pre["onesel"],
        })

    trace = bool(int(os.environ.get("KERNEL_TRACE", "0")))
    res = run_bass_kernel_spmd(nc, in_maps, core_ids=list(range(NCORES)), trace=trace)
    LAST_EXEC_NS = res.exec_time_ns

    sizes = pre["sizes"].astype(np.float32)
    gsum = np.concatenate([res.results[c]["gsum"] for c in range(NCORES)], axis=0)
    gmaxs = np.concatenate([res.results[c]["gmax"].T for c in range(NCORES)], axis=0)
    seqs = np.concatenate([res.results[c]["seqsum"] for c in range(NCORES)], axis=0)

    mean_pool = gsum / sizes[:, None]
    g = np.concatenate([mean_pool, gmaxs], axis=1)
    graph_logit = _mlp(g, params["mlp"])
    seq_mean = seqs / float(L)
    logits = _mlp(seq_mean, params["mlp2"])
    return (_sigmoid(logits + graph_logit).astype(np.float32),
            _sigmoid(graph_logit).astype(np.float32),
            _sigmoid(logits).astype(np.float32))


# revision 14
# speedup vs baseline: 1.0278x; 1.0278x over previous
import os
import sys

sys.path.insert(0, "/opt/trn_rl_repo")

import numpy as np

import concourse.bacc as bacc
import concourse.bass as bass
import concourse.mybir as mybir
import concourse.tile as tile
from concourse.bass_utils import run_bass_kernel_spmd
from concourse.masks import make_identity

FP32 = mybir.dt.float32
BF16 = mybir.dt.bfloat16
I32 = mybir.dt.int32

P = 128
NCORES = 8
B = 32
GP = 768            # padded slots per graph
GPC = 4             # graphs per core
SPC = GP * GPC      # 3072 slots per core
NT = SPC // P       # 24 node tiles per core
NSLOT = NCORES * SPC  # 24576
N = 20000
EMB = 768
L = 512

LAST_EXEC_NS = None

# table column layouts (bf16): [h | a_src | a_dst]
C1H, C1A = 1024, 8   # L1: 8 groups x 128
C2H, C2A = 512, 2    # L2: 2 groups x 256
C3H, C3A = 256, 2    # L3: 2 groups x 128
C1 = C1H + 2 * C1A   # 1040
C2 = C2H + 2 * C2A   # 516
C3 = C3H + 2 * C3A   # 260


def _fold_aug(Wcat, att_srcs, att_dsts, heads, cdim):
    # Wcat [cin, G*cdim]; att lists per conv of [heads, cdim]
    cin = Wcat.shape[0]
    G = Wcat.shape[1] // cdim
    asrc = np.zeros((cin, G), np.float32)
    adst = np.zeros((cin, G), np.float32)
    nconv = G // heads
    for conv in range(nconv):
        Wc = Wcat[:, conv * heads * cdim:(conv + 1) * heads * cdim].reshape(cin, heads, cdim)
        asrc[:, conv * heads:(conv + 1) * heads] = np.einsum("ihc,hc->ih", Wc, att_srcs[conv])
        adst[:, conv * heads:(conv + 1) * heads] = np.einsum("ihc,hc->ih", Wc, att_dsts[conv])
    return np.concatenate([Wcat, asrc, adst], axis=1).astype(np.float32)


def preprocess(x, edge_index, batch, params):
    x = np.asarray(x, np.float32)
    ei = np.asarray(edge_index)
    batch = np.asarray(batch).astype(np.int64)
    n = x.shape[0]

    sizes = np.bincount(batch, minlength=B)
    assert sizes.max() <= GP, f"graph size {sizes.max()} > {GP}"
    starts = np.zeros(B, np.int64)
    starts[1:] = np.cumsum(sizes)[:-1]
    # batch is sorted; slot of node i = batch[i]*GP + (i - start[batch[i]])
    slot = batch * GP + (np.arange(n) - starts[batch])

    src = np.concatenate([ei[0], np.arange(n)]).astype(np.int64)
    dst = np.concatenate([ei[1], np.arange(n)]).astype(np.int64)
    ss = slot[src]
    ds = slot[dst]

    order = np.argsort(ds, kind="stable")
    ss, ds = ss[order], ds[order]
    core = ds // SPC
    tloc = (ds % SPC) // P

    # count per (core, tile)
    cnt = np.zeros((NCORES, NT), np.int64)
    np.add.at(cnt, (core, tloc), 1)
    max_cpt = int(np.ceil(cnt.max() / P))
    nchunk = NT * max_cpt

    srcg = np.zeros((NCORES, nchunk * P), np.int32)
    dstg = np.zeros((NCORES, nchunk * P), np.int32)
    dstl = np.full((NCORES, nchunk * P), 300.0, np.float32)

    # edges are sorted by ds -> contiguous runs per (core, tile)
    # compute position of each edge within its (core,tile) run
    key = core * NT + tloc
    # start index of each key run
    run_start = np.zeros(NCORES * NT, np.int64)
    ksorted_counts = np.bincount(key, minlength=NCORES * NT)
    run_start[1:] = np.cumsum(ksorted_counts)[:-1]
    pos_in_run = np.arange(len(ss)) - run_start[key]
    for c in range(NCORES):
        m = core == c
        pp = (tloc[m] * (max_cpt * P) + pos_in_run[m]).astype(np.int64)
        srcg[c, pp] = ss[m]
        dstg[c, pp] = ds[m]
        dstl[c, pp] = (ds[m] % P).astype(np.float32)

    # [128, nchunk] column-major per chunk: element e of chunk k at [e, k]
    def cols(a):
        return np.ascontiguousarray(a.reshape(NCORES, nchunk, P).transpose(0, 2, 1))

    srcg_c, dstg_c, dstl_c = cols(srcg), cols(dstg), cols(dstl)

    # x slices transposed [256, SPC] per core
    xT = np.zeros((NCORES, x.shape[1], SPC), np.float32)
    for c in range(NCORES):
        m = (slot >= c * SPC) & (slot < (c + 1) * SPC)
        xT[c][:, slot[m] - c * SPC] = x[m].T

    convs = [params["convA"], params["convB"], params["convC"], params["convD"]]
    W1cat = np.concatenate([np.asarray(p["W"], np.float32) for p in convs], axis=1)
    w1aug = _fold_aug(W1cat, [np.asarray(p["att_src"], np.float32) for p in convs],
                      [np.asarray(p["att_dst"], np.float32) for p in convs], 2, 128)
    p2 = params["conv2"]
    w2aug = _fold_aug(np.asarray(p2["W"], np.float32), [np.asarray(p2["att_src"], np.float32)],
                      [np.asarray(p2["att_dst"], np.float32)], 2, 256)
    p3 = params["conv3"]
    w3aug = _fold_aug(np.asarray(p3["W"], np.float32), [np.asarray(p3["att_src"], np.float32)],
                      [np.asarray(p3["att_dst"], np.float32)], 2, 128)

    b1 = np.concatenate([np.asarray(p["b"], np.float32) for p in convs])
    b1t = np.tile(b1[None, :], (P, 1)).astype(np.float32)
    b2t = np.tile(np.asarray(p2["b"], np.float32)[None, :], (P, 1))
    b3t = np.tile(np.asarray(p3["b"], np.float32)[None, :], (P, 1))

    iota = np.tile(np.arange(P, dtype=np.float32)[None, :], (P, 1))

    onehot = np.zeros((NCORES, SPC, GPC), np.float32)
    padmask = np.full((NCORES, P, SPC), -1e30, np.float32)
    for c in range(NCORES):
        for gl in range(GPC):
            g = c * GPC + gl
            sz = int(sizes[g])
            onehot[c, gl * GP:gl * GP + sz, gl] = 1.0
            padmask[c, :, gl * GP:gl * GP + sz] = 0.0

    onesel = np.zeros((P, GPC * GPC), np.float32)
    for g in range(GPC):
        onesel[:, g * GPC + g] = 1.0

    return dict(slot=slot, sizes=sizes, max_cpt=max_cpt, nchunk=nchunk,
                srcg=srcg_c, dstg=dstg_c, dstl=dstl_c, xT=xT,
                w1aug=w1aug, w2aug=w2aug, w3aug=w3aug,
                b1t=b1t, b2t=b2t, b3t=b3t, iota=iota,
                onehot=onehot, padmask=padmask, onesel=onesel)


def build_program(max_cpt):
    nchunk = NT * max_cpt
    nc = bacc.Bacc("TRN2", num_devices=NCORES)

    xT = nc.dram_tensor("xT", [256, SPC], FP32, kind="ExternalInput")
    w1 = nc.dram_tensor("w1", [256, C1], FP32, kind="ExternalInput")
    w2 = nc.dram_tensor("w2", [512, C2], FP32, kind="ExternalInput")
    w3 = nc.dram_tensor("w3", [256, C3], FP32, kind="ExternalInput")
    b1t = nc.dram_tensor("b1t", [P, 512], FP32, kind="ExternalInput")
    b2t = nc.dram_tensor("b2t", [P, 256], FP32, kind="ExternalInput")
    b3t = nc.dram_tensor("b3t", [P, 128], FP32, kind="ExternalInput")
    iota = nc.dram_tensor("iota", [P, P], FP32, kind="ExternalInput")
    srcg = nc.dram_tensor("srcg", [P, nchunk], I32, kind="ExternalInput")
    dstg = nc.dram_tensor("dstg", [P, nchunk], I32, kind="ExternalInput")
    dstl = nc.dram_tensor("dstl", [P, nchunk], FP32, kind="ExternalInput")
    onehot = nc.dram_tensor("onehot", [SPC, GPC], FP32, kind="ExternalInput")
    padmask = nc.dram_tensor("padmask", [P, SPC], FP32, kind="ExternalInput")
    seq = nc.dram_tensor("seq", [GPC * L, EMB], FP32, kind="ExternalInput")
    onesel = nc.dram_tensor("onesel", [P, GPC * GPC], FP32, kind="ExternalInput")

    gsum = nc.dram_tensor("gsum", [GPC, 128], FP32, kind="ExternalOutput")
    gmax = nc.dram_tensor("gmax", [P, GPC], FP32, kind="ExternalOutput")
    seqsum = nc.dram_tensor("seqsum", [GPC, EMB], FP32, kind="ExternalOutput")

    slice1 = nc.dram_tensor("slice1", [SPC, C1], BF16)
    slice2 = nc.dram_tensor("slice2", [SPC, C2], BF16)
    slice3 = nc.dram_tensor("slice3", [SPC, C3], BF16)
    table1 = nc.dram_tensor("table1", [NSLOT, C1], BF16, addr_space="Shared")
    table2 = nc.dram_tensor("table2", [NSLOT, C2], BF16, addr_space="Shared")
    table3 = nc.dram_tensor("table3", [NSLOT, C3], BF16, addr_space="Shared")

    RG = [list(range(NCORES))]
    AOT = mybir.AluOpType

    with tile.TileContext(nc) as tc:
        with tc.tile_pool(name="const", bufs=1) as cp:
            ident = cp.tile([P, P], FP32)
            make_identity(nc, ident[:])
            iota_sb = cp.tile([P, P], FP32)
            nc.sync.dma_start(out=iota_sb[:], in_=iota[:])
            srcg_sb = cp.tile([P, nchunk], I32)
            nc.sync.dma_start(out=srcg_sb[:], in_=srcg[:])
            dstg_sb = cp.tile([P, nchunk], I32)
            nc.sync.dma_start(out=dstg_sb[:], in_=dstg[:])
            dstl_sb = cp.tile([P, nchunk], FP32)
            nc.sync.dma_start(out=dstl_sb[:], in_=dstl[:])
            b1_sb = cp.tile([P, 512], FP32)
            nc.sync.dma_start(out=b1_sb[:], in_=b1t[:])
            b2_sb = cp.tile([P, 256], FP32)
            nc.sync.dma_start(out=b2_sb[:], in_=b2t[:])
            b3_sb = cp.tile([P, 128], FP32)
            nc.sync.dma_start(out=b3_sb[:], in_=b3t[:])
            xT_sb = [cp.tile([P, SPC], FP32, tag=f"xT{k}", name=f"xT{k}") for k in range(2)]
            for k in range(2):
                nc.sync.dma_start(out=xT_sb[k][:], in_=xT[k * P:(k + 1) * P, :])
            w1_sb = [cp.tile([P, C1], FP32, tag=f"w1_{k}", name=f"w1s{k}") for k in range(2)]
            for k in range(2):
                nc.sync.dma_start(out=w1_sb[k][:], in_=w1[k * P:(k + 1) * P, :])
            w2_sb = [cp.tile([P, C2], FP32, tag=f"w2_{k}", name=f"w2s{k}") for k in range(4)]
            for k in range(4):
                nc.sync.dma_start(out=w2_sb[k][:], in_=w2[k * P:(k + 1) * P, :])
            w3_sb = [cp.tile([P, C3], FP32, tag=f"w3_{k}", name=f"w3s{k}") for k in range(2)]
            for k in range(2):
                nc.sync.dma_start(out=w3_sb[k][:], in_=w3[k * P:(k + 1) * P, :])
            x2T_sb = [cp.tile([P, SPC], FP32, tag=f"x2T{k}", name=f"x2T{k}") for k in range(4)]
            x3T_sb = [cp.tile([P, SPC], FP32, tag=f"x3T{k}", name=f"x3T{k}") for k in range(2)]
            hfT_sb = cp.tile([P, SPC], FP32)

            # ---------------- Phase A: h1_aug = x @ w1aug ----------------
            with tc.tile_pool(name="psA", bufs=2, space="PSUM") as psA, \
                 tc.tile_pool(name="sbA", bufs=3) as sbA:
                for t in range(NT):
                    ph = [psA.tile([P, 512], FP32, tag="pha", name="pha"), psA.tile([P, 512], FP32, tag="phb", name="phb"), psA.tile([P, 16], FP32, tag="phc", name="phc")]
                    fr = [(0, 512), (512, 1024), (1024, 1040)]
                    for pi, (f0, f1) in enumerate(fr):
                        for k in range(2):
                            nc.tensor.matmul(ph[pi][:, :f1 - f0],
                                             lhsT=xT_sb[k][:, t * P:(t + 1) * P],
                                             rhs=w1_sb[k][:, f0:f1],
                                             start=(k == 0), stop=(k == 1))
                    hb = sbA.tile([P, C1], BF16)
                    for pi, (f0, f1) in enumerate(fr):
                        nc.vector.tensor_copy(out=hb[:, f0:f1], in_=ph[pi][:, :f1 - f0])
                    nc.sync.dma_start(out=slice1[t * P:(t + 1) * P, :], in_=hb[:])

            with tc.tile_pool(name="dr1", bufs=1, space="DRAM") as dr1:
                nc.gpsimd.collective_compute("AllGather", AOT.bypass, replica_groups=RG,
                                             ins=[slice1[:]], outs=[table1[:]])

            # ---------------- Phase B: edge L1 -> x2 -> h2_aug ----------------
            def edge_phase(tbl, CH, CA, ngroups, gw, pn_shapes, bias_sb, relu,
                           out_cb):
                # returns per-tile node-partition output [P, ngroups//2*gw]
                with tc.tile_pool(name="psE", bufs=2, space="PSUM") as psE, \
                     tc.tile_pool(name="psD", bufs=1, space="PSUM") as psD, \
                     tc.tile_pool(name="sbE", bufs=8) as sbE:
                    CT = CH + 2 * CA
                    for t in range(NT):
                        pns = [psE.tile([P, s], FP32, tag=f"pn{i}", name=f"pn{i}")
                               for i, s in enumerate(pn_shapes)]
                        pd = psD.tile([P, CA], FP32, tag="pd")
                        last = max_cpt - 1
                        for j in range(max_cpt):
                            kk = t * max_cpt + j
                            hbuf = sbE.tile([P, CT], BF16, tag="hbuf")
                            nc.gpsimd.indirect_dma_start(
                                out=hbuf[:], out_offset=None, in_=tbl[:, :],
                                in_offset=bass.IndirectOffsetOnAxis(ap=srcg_sb[:, kk:kk + 1], axis=0))
                            adst = sbE.tile([P, CA], BF16, tag="adst")
                            nc.gpsimd.indirect_dma_start(
                                out=adst[:], out_offset=None, in_=tbl[:, 0:CA],
                                in_offset=bass.IndirectOffsetOnAxis(ap=dstg_sb[:, kk:kk + 1], axis=0),
                                element_offset=CH + CA)
                            sv = sbE.tile([P, CA], FP32, tag="sv")
                            nc.vector.tensor_tensor(out=sv[:], in0=hbuf[:, CH:CH + CA],
                                                    in1=adst[:], op=AOT.add)
                            ev = sbE.tile([P, CA], FP32, tag="ev")
                            nc.vector.tensor_scalar(out=ev[:], in0=sv[:], scalar1=0.2,
                                                    scalar2=None, op0=AOT.mult)
                            nc.vector.tensor_tensor(out=ev[:], in0=ev[:], in1=sv[:], op=AOT.max)
                            pbf = sbE.tile([P, CA], BF16, tag="pbf")
                            nc.scalar.activation(pbf[:], ev[:], mybir.ActivationFunctionType.Exp)
                            S = sbE.tile([P, P], BF16, tag="S")
                            nc.vector.tensor_tensor(
                                out=S[:], in0=iota_sb[:],
                                in1=dstl_sb[:, kk:kk + 1].to_broadcast([P, P]),
                                op=AOT.is_equal)
                            for g in range(ngroups):
                                nc.vector.tensor_tensor(
                                    out=hbuf[:, g * gw:(g + 1) * gw],
                                    in0=hbuf[:, g * gw:(g + 1) * gw],
                                    in1=pbf[:, g:g + 1].to_broadcast([P, gw]),
                                    op=AOT.mult)
                            nc.tensor.matmul(pd[:], lhsT=S[:], rhs=pbf[:],
                                             start=(j == 0), stop=(j == last))
                            gpb = pn_shapes[0] // gw  # groups per psum tile
                            for g in range(ngroups):
                                pt = pns[g // gpb]
                                off = (g % gpb) * gw
                                nc.tensor.matmul(pt[:, off:off + gw], lhsT=S[:],
                                                 rhs=hbuf[:, g * gw:(g + 1) * gw],
                                                 start=(j == 0), stop=(j == last))
                        # epilogue: alpha-normalize + head mean + bias (+relu)
                        den = sbE.tile([P, CA], FP32, tag="den")
                        nc.vector.tensor_scalar(out=den[:], in0=pd[:], scalar1=1e-16, scalar2=None, op0=AOT.add)
                        rden = sbE.tile([P, CA], FP32, tag="rden")
                        nc.vector.reciprocal(rden[:], den[:])
                        nc.vector.tensor_scalar(out=rden[:], in0=rden[:], scalar1=0.5,
                                                scalar2=None, op0=AOT.mult)
                        nconv = ngroups // 2
                        xo = sbE.tile([P, nconv * gw], FP32, tag="xo")
                        tmp = sbE.tile([P, gw], FP32, tag="tmp")
                        for conv in range(nconv):
                            g0, g1 = 2 * conv, 2 * conv + 1
                            gpb = pn_shapes[0] // gw

                            def pslice(g):
                                return pns[g // gpb][:, (g % gpb) * gw:(g % gpb) * gw + gw]

                            nc.vector.tensor_tensor(
                                out=tmp[:], in0=pslice(g0),
                                in1=rden[:, g0:g0 + 1].to_broadcast([P, gw]), op=AOT.mult)
                            nc.vector.tensor_tensor(
                                out=xo[:, conv * gw:(conv + 1) * gw], in0=pslice(g1),
                                in1=rden[:, g1:g1 + 1].to_broadcast([P, gw]), op=AOT.mult)
                            nc.vector.tensor_tensor(
                                out=xo[:, conv * gw:(conv + 1) * gw],
                                in0=xo[:, conv * gw:(conv + 1) * gw], in1=tmp[:], op=AOT.add)
                            nc.vector.tensor_tensor(
                                out=xo[:, conv * gw:(conv + 1) * gw],
                                in0=xo[:, conv * gw:(conv + 1) * gw],
                                in1=bias_sb[:, conv * gw:(conv + 1) * gw], op=AOT.add)
                            if relu:
                                nc.scalar.activation(xo[:, conv * gw:(conv + 1) * gw],
                                                     xo[:, conv * gw:(conv + 1) * gw],
                                                     mybir.ActivationFunctionType.Relu)
                        out_cb(t, xo, psE, sbE)

            # phase B body callback: transpose x2, compute h2_aug, write slice2
            with tc.tile_pool(name="psT", bufs=1, space="PSUM") as psT, \
                 tc.tile_pool(name="sbT", bufs=3) as sbT:

                def cb_b(t, xo, psE, sbE):
                    for fc in range(4):
                        ptr = psT.tile([P, P], FP32, tag="tr")
                        nc.tensor.transpose(out=ptr[:], in_=xo[:, fc * P:(fc + 1) * P],
                                            identity=ident[:])
                        nc.vector.tensor_copy(out=x2T_sb[fc][:, t * P:(t + 1) * P], in_=ptr[:])
                    pha = psT.tile([P, 512], FP32, tag="h2a")
                    phb = psT.tile([P, 4], FP32, tag="h2b")
                    for k in range(4):
                        nc.tensor.matmul(pha[:], lhsT=x2T_sb[k][:, t * P:(t + 1) * P],
                                         rhs=w2_sb[k][:, 0:512], start=(k == 0), stop=(k == 3))
                        nc.tensor.matmul(phb[:], lhsT=x2T_sb[k][:, t * P:(t + 1) * P],
                                         rhs=w2_sb[k][:, 512:516], start=(k == 0), stop=(k == 3))
                    hb2 = sbT.tile([P, C2], BF16, tag="hb2")
                    nc.vector.tensor_copy(out=hb2[:, 0:512], in_=pha[:])
                    nc.vector.tensor_copy(out=hb2[:, 512:516], in_=phb[:])
                    nc.sync.dma_start(out=slice2[t * P:(t + 1) * P, :], in_=hb2[:])

                edge_phase(table1, C1H, C1A, 8, 128, [512, 512], b1_sb, True, cb_b)

            nc.gpsimd.collective_compute("AllGather", AOT.bypass, replica_groups=RG,
                                         ins=[slice2[:]], outs=[table2[:]])

            # ---------------- Phase C: edge L2 -> x3 -> h3_aug ----------------
            with tc.tile_pool(name="psT3", bufs=1, space="PSUM") as psT3, \
                 tc.tile_pool(name="sbT3", bufs=3) as sbT3:

                def cb_c(t, xo, psE, sbE):
                    for fc in range(2):
                        ptr = psT3.tile([P, P], FP32, tag="tr3")
                        nc.tensor.transpose(out=ptr[:], in_=xo[:, fc * P:(fc + 1) * P],
                                            identity=ident[:])
                        nc.vector.tensor_copy(out=x3T_sb[fc][:, t * P:(t + 1) * P], in_=ptr[:])
                    pha = psT3.tile([P, C3], FP32, tag="h3a")
                    for k in range(2):
                        nc.tensor.matmul(pha[:], lhsT=x3T_sb[k][:, t * P:(t + 1) * P],
                                         rhs=w3_sb[k][:], start=(k == 0), stop=(k == 1))
                    hb3 = sbT3.tile([P, C3], BF16, tag="hb3")
                    nc.vector.tensor_copy(out=hb3[:], in_=pha[:])
                    nc.sync.dma_start(out=slice3[t * P:(t + 1) * P, :], in_=hb3[:])

                edge_phase(table2, C2H, C2A, 2, 256, [512], b2_sb, False, cb_c)

            nc.gpsimd.collective_compute("AllGather", AOT.bypass, replica_groups=RG,
                                         ins=[slice3[:]], outs=[table3[:]])

            # ---------------- Phase D: edge L3 -> h_f -> pooling ----------------
            with tc.tile_pool(name="psP", bufs=1, space="PSUM") as psP, \
                 tc.tile_pool(name="sbP", bufs=3) as sbP:
                ppool = psP.tile([GPC, 128], FP32, tag="pool")

                def cb_d(t, xo, psE, sbE):
                    oh = sbP.tile([P, GPC], FP32, tag="oh")
                    nc.sync.dma_start(out=oh[:], in_=onehot[t * P:(t + 1) * P, :])
                    nc.tensor.matmul(ppool[:], lhsT=oh[:], rhs=xo[:],
                                     start=(t == 0), stop=(t == NT - 1))
                    ptr = psE.tile([P, P], FP32, tag="trD")
                    nc.tensor.transpose(out=ptr[:], in_=xo[:], identity=ident[:])
                    nc.vector.tensor_copy(out=hfT_sb[:, t * P:(t + 1) * P], in_=ptr[:])

                edge_phase(table3, C3H, C3A, 2, 128, [256], b3_sb, False, cb_d)

                pm = sbP.tile([P, SPC], FP32, tag="pm")
                nc.sync.dma_start(out=pm[:], in_=padmask[:])
                nc.vector.tensor_tensor(out=hfT_sb[:], in0=hfT_sb[:], in1=pm[:], op=AOT.add)
                gm = sbP.tile([P, GPC], FP32, tag="gm")
                for g in range(GPC):
                    nc.vector.tensor_reduce(gm[:, g:g + 1], hfT_sb[:, g * GP:(g + 1) * GP],
                                            mybir.AxisListType.X, AOT.max)
                nc.sync.dma_start(out=gmax[:], in_=gm[:])
                gs = sbP.tile([GPC, 128], FP32, tag="gs")
                nc.vector.tensor_copy(out=gs[:], in_=ppool[:])
                nc.sync.dma_start(out=gsum[:], in_=gs[:])

            # ---------------- Phase E: seq mean ----------------
            with tc.tile_pool(name="psS", bufs=1, space="PSUM") as psS, \
                 tc.tile_pool(name="sbS", bufs=3) as sbS:
                os_sb = sbS.tile([P, GPC * GPC], FP32, tag="os")
                nc.sync.dma_start(out=os_sb[:], in_=onesel[:])
                psq = [psS.tile([GPC, 384], FP32, tag=f"sq{i}", name=f"psq{i}") for i in range(2)]
                nchk = GPC * L // P  # 16
                for ci in range(nchk):
                    st = sbS.tile([P, EMB], FP32, tag="st")
                    nc.sync.dma_start(out=st[:], in_=seq[ci * P:(ci + 1) * P, :])
                    g = ci // (L // P)
                    for h in range(2):
                        nc.tensor.matmul(psq[h][:], lhsT=os_sb[:, g * GPC:(g + 1) * GPC],
                                         rhs=st[:, h * 384:(h + 1) * 384],
                                         start=(ci == 0), stop=(ci == nchk - 1))
                sq = sbS.tile([GPC, EMB], FP32, tag="sq")
                nc.vector.tensor_copy(out=sq[:, 0:384], in_=psq[0][:])
                nc.vector.tensor_copy(out=sq[:, 384:768], in_=psq[1][:])
                nc.sync.dma_start(out=seqsum[:], in_=sq[:])

    nc.compile()
    return nc


def _mlp(x, p):
    h = x @ np.asarray(p["W1"], np.float32) + np.asarray(p["b1"], np.float32)
    h = np.maximum(h, 0.0)
    return h @ np.asarray(p["W2"], np.float32) + np.asarray(p["b2"], np.float32)


def _sigmoid(v):
    return 1.0 / (1.0 + np.exp(-v))


def kernel(x, edge_index, edge_attr, batch, seq_hidden, params):
    global LAST_EXEC_NS
    pre = preprocess(x, edge_index, batch, params)
    nc = build_program(pre["max_cpt"])

    seq_hidden = np.asarray(seq_hidden, np.float32)
    in_maps = []
    for c in range(NCORES):
        in_maps.append({
            "xT": np.ascontiguousarray(pre["xT"][c]),
            "w1": pre["w1aug"], "w2": pre["w2aug"], "w3": pre["w3aug"],
            "b1t": pre["b1t"], "b2t": pre["b2t"], "b3t": pre["b3t"],
            "iota": pre["iota"],
            "srcg": np.ascontiguousarray(pre["srcg"][c]),
            "dstg": np.ascontiguousarray(pre["dstg"][c]),
            "dstl": np.ascontiguousarray(pre["dstl"][c]),
            "onehot": np.ascontiguousarray(pre["onehot"][c]),
            "padmask": np.ascontiguousarray(pre["padmask"][c]),
            "seq": np.ascontiguousarray(
                seq_hidden[c * GPC:(c + 1) * GPC].reshape(GPC * L, EMB)),
            "onesel": pre["onesel"],
        })

    trace = bool(int(os.environ.get("KERNEL_TRACE", "0")))
    res = run_bass_kernel_spmd(nc, in_maps, core_ids=list(range(NCORES)), trace=trace)
    LAST_EXEC_NS = res.exec_time_ns

    sizes = pre["sizes"].astype(np.float32)
    gsum = np.concatenate([res.results[c]["gsum"] for c in range(NCORES)], axis=0)
    gmaxs = np.concatenate([res.results[c]["gmax"].T for c in range(NCORES)], axis=0)
    seqs = np.concatenate([res.results[c]["seqsum"] for c in range(NCORES)], axis=0)

    mean_pool = gsum / sizes[:, None]
    g = np.concatenate([mean_pool, gmaxs], axis=1)
    graph_logit = _mlp(g, params["mlp"])
    seq_mean = seqs / float(L)
    logits = _mlp(seq_mean, params["mlp2"])
    return (_sigmoid(logits + graph_logit).astype(np.float32),
            _sigmoid(graph_logit).astype(np.float32),
            _sigmoid(logits).astype(np.float32))
